# revision 4
# baseline (speedup 1.0000x reference)
"""Trainium2 Bass kernel for nn_DensityVQC (batched 2-qubit VQC Z-expectation).

Algebra
-------
The reference builds rho_b = conj(psi_b) psi_b^T (note: transpose of the
standard density matrix), evolves rho' = U rho U^dag and returns
tr(rho' Z0) with Z0 = diag(1,1,-1,-1).  This collapses to a per-row
quadratic form: with V = conj(U) (the transposed-rho convention flips the
conjugation) and phi = V psi,

    out_b = |phi_0|^2 + |phi_1|^2 - |phi_2|^2 - |phi_3|^2
          = 2 * || C psi_b ||^2 - ||psi_b||^2        (C = V[0:2, :], U unitary)
          = || A r_b + B m_b ||^2 - 1                (inputs are unit-norm)

with real 4x4 matrices A = sqrt(2)*[Re C; Im C], B = sqrt(2)*[-Im C; Re C].
So the device kernel is: per batch row (r, m in R^4), compute w = A r + B m,
then out = sum(w^2) - 1.  No [B,4,4] density matrices are ever materialized.

Device mapping (per core, pure data parallel over 8 cores)
----------------------------------------------------------
Per-core input slice: 131072 rows x 4 f32 for r and m, loaded as resident
[128, 4096] SBUF tiles (partition p holds rows [1024p, 1024p+1024)).
Processed in 8 "supertiles" of 512 free columns (16384 rows):
  1. 8x PE transpose [128,128] (fp32, exact) -> component-major tiles in PSUM
  2. copy PSUM->SBUF (DVE for r, ACT for m)
  3. projection: phi = blkdiag32(A^T)^T . Rt + blkdiag32(B^T)^T . Mt as two
     accumulating float32r matmuls (full PE rate at N=512)
  4. ACT Square: S = phi^2 -> SBUF
  5. 4x fused reduce+untranspose matmuls: stationary = S slice, moving =
     group-sum pattern [128,32]; output lands batch-contiguous in PSUM
  6. DVE: out = S_reduced - 1 -> resident output tile
Output [128, 1024] DMAs out fully contiguous (b = 1024p + col).
"""

import sys
import numpy as np

if "/opt/trn_rl_repo" not in sys.path:
    sys.path.insert(0, "/opt/trn_rl_repo")

import concourse.bass as bass
import concourse.tile as tile
from concourse import bacc, mybir
from concourse import bass_utils

N_CORES = 8
BSZ = 1_048_576
BC = BSZ // N_CORES            # 131072 rows per core
COLS = BC * 4 // 128           # 4096 free cols of resident [128, COLS] inputs
OUT_COLS = COLS // 4           # 1024
N_ST = COLS // 512             # 8 supertiles
F32 = mybir.dt.float32
F32R = mybir.dt.float32r
N_LAYERS = 6


def _circuit_unitary(ry, rz):
    """4x4 circuit unitary, float64 mirror of reference._circuit_unitary."""
    ry = np.asarray(ry, dtype=np.float64)
    rz = np.asarray(rz, dtype=np.float64)
    cnot = np.array(
        [[1, 0, 0, 0], [0, 1, 0, 0], [0, 0, 0, 1], [0, 0, 1, 0]],
        dtype=np.complex128,
    )

    def _ry(th):
        c, s = np.cos(th / 2), np.sin(th / 2)
        return np.array([[c, -s], [s, c]], dtype=np.complex128)

    def _rz(th):
        return np.diag([np.exp(-0.5j * th), np.exp(0.5j * th)])

    u = np.eye(4, dtype=np.complex128)
    for l in range(ry.shape[0]):
        ry_full = np.kron(_ry(ry[l, 0]), _ry(ry[l, 1]))
        rz_full = np.kron(_rz(rz[l, 0]), _rz(rz[l, 1]))
        u = cnot @ (rz_full @ (ry_full @ u))
    return u


def _host_consts(ry_params, rz_params):
    u = _circuit_unitary(ry_params, rz_params)
    c = np.conj(u)[0:2, :]
    a = np.sqrt(2.0) * np.vstack([c.real, c.imag])     # 4x4, w = A r + B m
    b = np.sqrt(2.0) * np.vstack([-c.imag, c.real])
    eye32 = np.eye(32, dtype=np.float32)
    # lhsT[k=4g+c, m=4g+j] = A[j, c]  ->  block_diag of A.T
    ablk = np.kron(eye32, a.T.astype(np.float32)).astype(np.float32)
    bblk = np.kron(eye32, b.T.astype(np.float32)).astype(np.float32)
    zsum = np.kron(eye32, np.ones((4, 1), dtype=np.float32)).astype(np.float32)
    ident = np.eye(128, dtype=np.float32)
    return ident, ablk, bblk, zsum


def _build_program():
    nc = bacc.Bacc("TRN2", target_bir_lowering=False, debug=False)
    sr_d = nc.dram_tensor("sr", [128, COLS], F32, kind="ExternalInput")
    si_d = nc.dram_tensor("si", [128, COLS], F32, kind="ExternalInput")
    ident_d = nc.dram_tensor("ident", [128, 128], F32, kind="ExternalInput")
    ablk_d = nc.dram_tensor("ablk", [128, 128], F32R, kind="ExternalInput")
    bblk_d = nc.dram_tensor("bblk", [128, 128], F32R, kind="ExternalInput")
    zsum_d = nc.dram_tensor("zsum", [128, 32], F32, kind="ExternalInput")
    out_d = nc.dram_tensor("out", [128, OUT_COLS], F32, kind="ExternalOutput")

    with tile.TileContext(nc) as tc:
        with (
            tc.tile_pool(name="const", bufs=1) as cpool,
            tc.tile_pool(name="io", bufs=1) as iopool,
            tc.tile_pool(name="work", bufs=3) as wpool,
            tc.tile_pool(name="psum", bufs=2, space=bass.MemorySpace.PSUM) as ppool,
        ):
            ident = cpool.tile([128, 128], F32, name="ident_t")
            ablk = cpool.tile([128, 128], F32R, name="ablk_t")
            bblk = cpool.tile([128, 128], F32R, name="bblk_t")
            zsum = cpool.tile([128, 32], F32, name="zsum_t")
            nc.sync.dma_start(ident[:], ident_d.ap())
            nc.sync.dma_start(ablk[:], ablk_d.ap())
            nc.sync.dma_start(bblk[:], bblk_d.ap())
            nc.sync.dma_start(zsum[:], zsum_d.ap())

            r_big = iopool.tile([128, COLS], F32, name="r_big")
            m_big = iopool.tile([128, COLS], F32, name="m_big")
            out_full = iopool.tile([128, OUT_COLS], F32, name="out_full")

            for st in range(N_ST):
                cs = bass.ts(st, 512)
                nc.sync.dma_start(r_big[:, cs], sr_d.ap()[:, cs])
                nc.sync.dma_start(m_big[:, cs], si_d.ap()[:, cs])

            for st in range(N_ST):
                rt_ps = ppool.tile([128, 512], F32, name="rt_ps")
                mt_ps = ppool.tile([128, 512], F32, name="mt_ps")
                for t in range(4):
                    src = bass.ds(512 * st + 128 * t, 128)
                    nc.tensor.transpose(
                        rt_ps[:, bass.ts(t, 128)], r_big[:, src], ident[:]
                    )
                for t in range(4):
                    src = bass.ds(512 * st + 128 * t, 128)
                    nc.tensor.transpose(
                        mt_ps[:, bass.ts(t, 128)], m_big[:, src], ident[:]
                    )

                rt_w = wpool.tile([128, 512], F32R, name="rt_w")
                mt_w = wpool.tile([128, 512], F32R, name="mt_w")
                nc.vector.tensor_copy(rt_w[:], rt_ps[:])
                nc.scalar.copy(mt_w[:], mt_ps[:])

                phi = ppool.tile([128, 512], F32, name="phi")
                nc.tensor.matmul(
                    phi[:], ablk[:], rt_w[:], start=True, stop=False
                )
                nc.tensor.matmul(
                    phi[:], bblk[:], mt_w[:], start=False, stop=True
                )

                s_sb = wpool.tile([128, 512], F32, name="s_sb")
                nc.scalar.activation(
                    s_sb[:], phi[:], mybir.ActivationFunctionType.Square
                )

                outb = ppool.tile([128, 128], F32, name="outb")
                for j2 in range(4):
                    nc.tensor.matmul(
                        outb[:, bass.ts(j2, 32)],
                        s_sb[:, bass.ts(j2, 128)],
                        zsum[:],
                    )
                nc.vector.tensor_scalar_add(
                    out_full[:, bass.ts(st, 128)], outb[:], -1.0
                )

            nc.sync.dma_start(out_d.ap(), out_full[:])
    nc.compile()
    return nc


_PROG_CACHE = None


def _get_program():
    global _PROG_CACHE
    if _PROG_CACHE is None:
        _PROG_CACHE = _build_program()
    return _PROG_CACHE


def _run(ry_params, rz_params, states_real, states_imag, **hw_kwargs):
    ident, ablk, bblk, zsum = _host_consts(ry_params, rz_params)
    states_real = np.ascontiguousarray(states_real, dtype=np.float32)
    states_imag = np.ascontiguousarray(states_imag, dtype=np.float32)
    in_maps = []
    for k in range(N_CORES):
        sl = slice(k * BC, (k + 1) * BC)
        in_maps.append(
            {
                "sr": states_real[sl].reshape(128, COLS),
                "si": states_imag[sl].reshape(128, COLS),
                "ident": ident,
                "ablk": ablk,
                "bblk": bblk,
                "zsum": zsum,
            }
        )
    nc = _get_program()
    res = bass_utils.run_bass_kernel_spmd(
        nc, in_maps, core_ids=list(range(N_CORES)), **hw_kwargs
    )
    out = np.concatenate(
        [res.results[k]["out"].reshape(-1) for k in range(N_CORES)]
    ).astype(np.float32)
    return out, res


def kernel(ry_params, rz_params, states_real, states_imag):
    out, _ = _run(ry_params, rz_params, states_real, states_imag)
    return out
